# revision 17
# baseline (speedup 1.0000x reference)
"""CMaxPool4d (complex modulus max-pool, K=2 stride 2 over 4 spatial dims) on 8 Trainium2 cores.

Input  : [8, 2, 32, 16, 16, 16, 16] f32  (dim1 = real/imag)
Output : [8, 2, 32, 8, 8, 8, 8] f32      (value of r/i at the max-|z| position of each 2^4 window)

Strategy (data-parallel over batch, core b <- batch b), measured ~61us/iter
(vs ~86us for the previous all-f32 DVE-heavy version):
- m = r^2 + i^2 off the DVE: ACT Square pass + GPSIMD (Pool) adds
  (K_M_ENG=ap default; =dve uses one custom DVE op sq(Src0)+sq(Src1)).
  sqrt is monotone, so it is not needed for the argmax comparison.
- Payload tournament runs on a bf16 copy with (r,i) packed as ADJACENT
  pairs, so copy_predicated moves one u32 per merge instead of two f32 —
  halves the dominant DVE cost. Masks apply 1:1 (no ri broadcast).
  Output values are bf16-rounded (rel err ~1.7e-3, under the 2e-2 gate);
  all comparisons stay exact f32 on the DVE.
- 4-level pairwise tournament, LSB-first (d4, d3, d2, d1), strict is_gt so
  ties resolve to the lowest window index, matching jnp.argmax.
- The two independent half-chains per chunk are interleaved in program
  order so consecutive same-engine instructions have no data dependence.
- Pool (GPSIMD) on this toolchain only supports add/sub/mult/copy tensor
  ops (no max/is_gt), so comparisons cannot be offloaded there.

Host pre-permute (free): per chunk of 8 channels, partition p=(c8,hi),
free f = a*4096 + b*2048 + ri*1024 + d*512 + s*256 + q*64 + o3*8 + o4.
"""

import os
import sys

import numpy as np

for p in ("/opt/trn_rl_repo", "/opt/pypackages", "/root/.axon_site", "/root/.axon_site/_ro/trn_rl_repo", "/root/.axon_site/_ro/pypackages"):
    if os.path.isdir(p) and p not in sys.path:
        sys.path.append(p)

from concourse import bacc, mybir  # noqa: E402
from concourse.tile import TileContext  # noqa: E402
from concourse.bass_utils import run_bass_kernel_spmd  # noqa: E402

N_CORES = 8
RI = 2
C = 32
D = 16
O = D // 2
NCH = 8                    # channels per chunk
NCHUNK = C // NCH          # 4
SLAB = 1024                # free elems per slab per partition
XF = 8 * SLAB              # 8192

F32 = mybir.dt.float32
BF16 = mybir.dt.bfloat16
U16 = mybir.dt.uint16
I32 = mybir.dt.int32

# engine assignment: comma lists over levels L0,L1,L2,L3 (isgt) / L0,L1,L2 (max)
ISGT_ENG = os.environ.get("K_ISGT_ENG", "v,v,v,v").split(",")
MAX_ENG = os.environ.get("K_MAX_ENG", "v,v,v").split(",")
M_ENG = os.environ.get("K_M_ENG", "ap")  # dve (custom sq+sq) | ap (ACT squares + Pool adds)
SPLIT_DMA = int(os.environ.get("K_SPLIT_DMA", "2"))
BUFS = int(os.environ.get("K_BUFS", "2"))
BUFS_X = int(os.environ.get("K_BUFS_X", str(BUFS)))
LOOPS = int(os.environ.get("K_LOOPS", "1"))       # whole-kernel idempotent repeats (bench)
REP_DVE = int(os.environ.get("K_REP_DVE", "1"))
REP_ACT = int(os.environ.get("K_REP_ACT", "1"))
REP_GP = int(os.environ.get("K_REP_GP", "1"))
REP_DMA = int(os.environ.get("K_REP_DMA", "1"))
PAYLOAD = os.environ.get("K_PAYLOAD", "bf16")     # bf16 | f32 (f32 = no cast, exact)

_COMPILED = None
_SQSUM = None


def _get_sqsum_op():
    """Register (once per process) the custom DVE op m = in0^2 + in1^2."""
    global _SQSUM
    if _SQSUM is not None:
        return _SQSUM
    from concourse import dve_ops
    from concourse.dve_spec import Spec, Src0, Src1, sq, lower
    from concourse.dve_uop import DveOpSpec

    name = "SQ_SUM_ANT_K"
    if name in dve_ops._SUB_OPCODE_FOR_NAME:
        _SQSUM = next(op for op in dve_ops.OPS if op.name == name)
        return _SQSUM
    spec = Spec(
        body=sq(Src0) + sq(Src1),
        reference=lambda in0, in1, s0, s1, imm2: (
            in0.astype(np.float32) ** 2 + in1.astype(np.float32) ** 2
        ),
    )
    shas = {}
    for ver in ("v3", "v4"):
        uops = lower(spec, ver=ver)
        shas[ver] = DveOpSpec(name=name, opcode=31, uops=uops, rd1_en=True).sha(ver)
    op = dve_ops.DveOp(name, spec, subdim=False, uops_sha=shas)
    row = dve_ops._CUSTOM_DVE_ROW_BASE + len(dve_ops.OPS)
    assert row < 0x20
    dve_ops.OPS.append(op)
    dve_ops.CUSTOM_DVE_SPECS[name] = op.spec
    dve_ops._SUB_OPCODE_FOR_NAME[name] = row
    _SQSUM = op
    return op


def _eng(nc, which):
    return nc.vector if which.strip().startswith("v") else nc.gpsimd


def _build():
    sqsum = _get_sqsum_op()
    nc = bacc.Bacc("TRN2", num_devices=N_CORES)
    x_dram = nc.declare_dram_parameter("x", [NCHUNK, 128, XF], F32, isOutput=False)
    y_dram = nc.declare_dram_parameter(
        "y", [NCHUNK, 128, 512], BF16 if PAYLOAD == "bf16" else F32, isOutput=True
    )

    pay_dt = BF16 if PAYLOAD == "bf16" else F32

    from contextlib import ExitStack
    with TileContext(nc) as tc, ExitStack() as stack:
        pool_x = stack.enter_context(tc.tile_pool(name="xpool", bufs=BUFS_X))
        pool = stack.enter_context(tc.tile_pool(name="sbuf", bufs=BUFS))
        pool_mask = stack.enter_context(tc.tile_pool(name="maskpool", bufs=BUFS))
        for k in [kk for _ in range(LOOPS) for kk in range(NCHUNK)]:
            X = pool_x.tile([128, XF], F32, tag="X")
            for _ in range(REP_DMA):
                step = XF // SPLIT_DMA
                for j in range(SPLIT_DMA):
                    nc.sync.dma_start(
                        out=X[:, j * step:(j + 1) * step],
                        in_=x_dram[k][:, j * step:(j + 1) * step],
                    )

            xtr = X.rearrange("p (t ri f) -> p t ri f", t=4, ri=2)
            # payload copy (bf16, r/i interleaved pairs) and modulus tiles
            if PAYLOAD == "bf16":
                XB = pool.tile([128, XF], BF16, tag="XB")
                # XB physical layout [t, f, ri]; write via strided out AP so
                # in-order (t, ri, f) input lands interleaved
                xb_w = XB.rearrange("p (t f ri) -> p t ri f", t=4, ri=2)
                for h in range(2):
                    for _ in range(REP_ACT):
                        nc.scalar.activation(
                            xb_w[:, 2 * h:2 * h + 2],
                            X[:, h * 4096:(h + 1) * 4096],
                            mybir.ActivationFunctionType.Copy,
                        )
                # u32 view: one element per (r,i) pair
                xbp = XB.bitcast(mybir.dt.uint32).rearrange("p (t f) -> p t f", t=4)
            else:
                xbr = xtr
            M = pool.tile([128, 4096], F32, tag="M")  # 4 t-blocks of 1024
            mt = M.rearrange("p (t f) -> p t f", t=4)

            # m = r^2 + i^2 per half
            if M_ENG == "dve":
                # one custom DVE op per half
                for h in range(2):
                    t0 = 2 * h
                    for _ in range(REP_DVE):
                        nc.vector._custom_dve(
                            sqsum,
                            out=mt[:, t0:t0 + 2, :],
                            in0=xtr[:, t0:t0 + 2, 0, :],
                            in1=xtr[:, t0:t0 + 2, 1, :],
                        )
            else:
                # ACT squares (ri-major regrouped) + Pool adds, off the DVE
                for h in range(2):
                    SQ = pool.tile([128, 4096], F32, tag="SQ")
                    for _ in range(REP_ACT):
                        nc.scalar.activation(
                            SQ.rearrange("p (ri b f) -> p b ri f", ri=2, b=2),
                            X[:, h * 4096:(h + 1) * 4096],
                            mybir.ActivationFunctionType.Square,
                        )
                    for _ in range(REP_GP):
                        nc.gpsimd.tensor_tensor(
                            M[:, h * 2048:(h + 1) * 2048],
                            SQ[:, 0:2048], SQ[:, 2048:4096],
                            mybir.AluOpType.add,
                        )

            def pred(t0, n_t, mask_ap, half):
                """payload[t0..t0+n_t-1][0:half] <- [half:2*half] where mask."""
                if mask_ap.dtype == F32:
                    # Pool-made masks are f32 0.0/1.0; cp needs an int dtype —
                    # bitcast (0x0 / 0x3F800000) preserves nonzero-ness.
                    mask_ap = mask_ap.bitcast(I32)
                mk = mask_ap.rearrange("p (t f) -> p t f", t=n_t)
                for _ in range(REP_DVE):
                    if PAYLOAD == "bf16":
                        nc.vector.copy_predicated(
                            xbp[:, t0:t0 + n_t, 0:half], mk,
                            xbp[:, t0:t0 + n_t, half:2 * half],
                        )
                    else:
                        mkb = mk.unsqueeze(2).broadcast_to((128, n_t, 2, half))
                        nc.vector.copy_predicated(
                            xbr[:, t0:t0 + n_t, :, 0:half], mkb,
                            xbr[:, t0:t0 + n_t, :, half:2 * half],
                        )

            def level43(t0, n_t, half, lvl, tag):
                """one d4/d3-style level on blocks t0..t0+n_t-1: [0:half) vs [half:2half)."""
                ei = _eng(nc, ISGT_ENG[lvl])
                # Pool TT ops need matching dtypes: all-f32 is_gt (mask 0.0/1.0)
                maskT = pool_mask.tile(
                    [128, n_t * half], F32 if ei is nc.gpsimd else U16, tag=tag
                )
                if ei is nc.gpsimd:
                    # flat free APs per t-block for gpsimd
                    mkv = maskT.rearrange("p (t f) -> p t f", t=n_t)
                    for t in range(n_t):
                        for _ in range(REP_GP):
                            nc.gpsimd.tensor_tensor(
                                mkv[:, t], mt[:, t0 + t, half:2 * half],
                                mt[:, t0 + t, 0:half], mybir.AluOpType.is_gt,
                            )
                else:
                    for _ in range(REP_DVE):
                        nc.vector.tensor_tensor(
                            maskT.rearrange("p (t f) -> p t f", t=n_t),
                            mt[:, t0:t0 + n_t, half:2 * half],
                            mt[:, t0:t0 + n_t, 0:half],
                            mybir.AluOpType.is_gt,
                        )
                pred(t0, n_t, maskT, half)
                em = _eng(nc, MAX_ENG[lvl])
                if em is nc.gpsimd:
                    for t in range(n_t):
                        for _ in range(REP_GP):
                            nc.gpsimd.tensor_tensor(
                                mt[:, t0 + t, 0:half], mt[:, t0 + t, 0:half],
                                mt[:, t0 + t, half:2 * half], mybir.AluOpType.max,
                            )
                else:
                    for _ in range(REP_DVE):
                        nc.vector.tensor_tensor(
                            mt[:, t0:t0 + n_t, 0:half], mt[:, t0:t0 + n_t, 0:half],
                            mt[:, t0:t0 + n_t, half:2 * half], mybir.AluOpType.max,
                        )

            def level2(t0):
                """b-merge: block t0+1 into t0 on [0:256)."""
                ei = _eng(nc, ISGT_ENG[2])
                mask2 = pool_mask.tile(
                    [128, 256], F32 if ei is nc.gpsimd else U16, tag="mask2"
                )
                for _ in range(REP_GP if ei is nc.gpsimd else REP_DVE):
                    ei.tensor_tensor(
                        mask2[:, :], mt[:, t0 + 1, 0:256], mt[:, t0, 0:256],
                        mybir.AluOpType.is_gt,
                    )
                mk2 = mask2.bitcast(I32) if mask2.dtype == F32 else mask2
                for _ in range(REP_DVE):
                    if PAYLOAD == "bf16":
                        nc.vector.copy_predicated(
                            xbp[:, t0, 0:256], mk2, xbp[:, t0 + 1, 0:256]
                        )
                    else:
                        mk = mk2.unsqueeze(1).broadcast_to((128, 2, 256))
                        nc.vector.copy_predicated(
                            xbr[:, t0, :, 0:256], mk, xbr[:, t0 + 1, :, 0:256]
                        )
                em = _eng(nc, MAX_ENG[2])
                for _ in range(REP_GP if em is nc.gpsimd else REP_DVE):
                    em.tensor_tensor(
                        mt[:, t0, 0:256], mt[:, t0, 0:256], mt[:, t0 + 1, 0:256],
                        mybir.AluOpType.max,
                    )

            # interleave the two halves' chains: consecutive same-engine
            # instructions are independent, filling dependency gaps
            for half, tag in ((512, "mask4"), (256, "mask3")):
                lvl = 0 if half == 512 else 1
                for h in range(2):
                    level43(2 * h, 2, half, lvl, tag + str(h))
            for h in range(2):
                level2(2 * h)

            # D1 (a pairs: t=2 into t=0); no m update
            ei = _eng(nc, ISGT_ENG[3])
            mask1 = pool_mask.tile(
                [128, 256], F32 if ei is nc.gpsimd else U16, tag="mask1"
            )
            for _ in range(REP_GP if ei is nc.gpsimd else REP_DVE):
                ei.tensor_tensor(
                    mask1[:, :], mt[:, 2, 0:256], mt[:, 0, 0:256], mybir.AluOpType.is_gt
                )
            mk1 = mask1.bitcast(I32) if mask1.dtype == F32 else mask1
            for _ in range(REP_DVE):
                if PAYLOAD == "bf16":
                    nc.vector.copy_predicated(xbp[:, 0, 0:256], mk1, xbp[:, 2, 0:256])
                else:
                    mk = mk1.unsqueeze(1).broadcast_to((128, 2, 256))
                    nc.vector.copy_predicated(xbr[:, 0, :, 0:256], mk, xbr[:, 2, :, 0:256])

            # store winners (bf16 y is upcast on host)
            if PAYLOAD == "bf16":
                nc.sync.dma_start(out=y_dram[k], in_=XB[:, 0:512])
            else:
                nc.sync.dma_start(out=y_dram[k], in_=xbr[:, 0, :, 0:256])

    nc.compile()
    return nc


def _get_nc():
    global _COMPILED
    if _COMPILED is None:
        _COMPILED = _build()
    return _COMPILED


def _prep_core(xb: np.ndarray) -> np.ndarray:
    """xb: [2, 32, 16,16,16,16] -> [4, 128, 8192] slab-packed, parity-split."""
    # [ri, chunk, c8, o1, a, o2, b, o3, s, o4, d]
    t = xb.reshape(RI, C // NCH, NCH, O, 2, O, 2, O, 2, O, 2)
    # -> [chunk, a, b, ri, c8, o1, o2, d, s, o3, o4]
    t = t.transpose(1, 4, 6, 0, 2, 3, 5, 10, 8, 7, 9)
    # merge (o1,o2) -> split (hi, q)
    t = t.reshape(C // NCH, 2, 2, RI, NCH, 16, 4, 2, 2, O, O)
    # -> [chunk, c8, hi, a, b, ri, d, s, q, o3, o4]
    t = t.transpose(0, 4, 5, 1, 2, 3, 7, 8, 6, 9, 10)
    return np.ascontiguousarray(t).reshape(C // NCH, 128, XF)


def _post_core(y: np.ndarray) -> np.ndarray:
    """y: [4, 128, 512] -> [2, 32, 8, 8, 8, 8]."""
    if PAYLOAD == "bf16":
        # [chunk, c8, hi, q*o3*o4, ri] (ri innermost: packed pairs)
        yk = y.reshape(C // NCH, NCH, 16, 4 * O * O, RI)
        out = yk.transpose(4, 0, 1, 2, 3).reshape(RI, C, 16, 4 * O * O)
        return out.reshape(RI, C, O, O, O, O)
    # [chunk, c8, hi, ri, q, o3o4]
    yk = y.reshape(C // NCH, NCH, 16, RI, 4, O * O)
    out = yk.transpose(3, 0, 1, 2, 4, 5).reshape(RI, C, 16 * 4, O * O)
    return out.reshape(RI, C, O, O, O, O)


def _run(inputs_x: np.ndarray, trace: bool = False):
    nc = _get_nc()
    in_maps = [{"x": _prep_core(inputs_x[b])} for b in range(N_CORES)]
    last_err = None
    for _attempt in range(3):
        try:
            res = run_bass_kernel_spmd(nc, in_maps, list(range(N_CORES)), trace=trace)
            break
        except Exception as e:  # wedged-device retries
            last_err = e
            if "UNRECOVERABLE" not in str(e) and "UNAVAILABLE" not in str(e):
                raise
    else:
        raise last_err
    outs = np.empty((N_CORES, RI, C, O, O, O, O), dtype=np.float32)
    for b in range(N_CORES):
        outs[b] = _post_core(res.results[b]["y"].astype(np.float32))
    return outs, res


def kernel(input: np.ndarray) -> np.ndarray:
    input = np.asarray(input, dtype=np.float32)
    outs, _ = _run(input)
    return outs


# revision 19
# speedup vs baseline: 1.2604x; 1.2604x over previous
"""CMaxPool4d (complex modulus max-pool, K=2 stride 2 over 4 spatial dims) on 8 Trainium2 cores.

Input  : [8, 2, 32, 16, 16, 16, 16] f32  (dim1 = real/imag)
Output : [8, 2, 32, 8, 8, 8, 8] f32      (value of r/i at the max-|z| position of each 2^4 window)

Strategy (data-parallel over batch, core b <- batch b), measured ~61us/iter
(vs ~86us for the previous all-f32 DVE-heavy version):
- m = r^2 + i^2 off the DVE: ACT Square pass + GPSIMD (Pool) adds
  (K_M_ENG=ap default; =dve uses one custom DVE op sq(Src0)+sq(Src1)).
  sqrt is monotone, so it is not needed for the argmax comparison.
- Payload tournament runs on a bf16 copy with (r,i) packed as ADJACENT
  pairs, so copy_predicated moves one u32 per merge instead of two f32 —
  halves the dominant DVE cost. Masks apply 1:1 (no ri broadcast).
  Output values are bf16-rounded (rel err ~1.7e-3, under the 2e-2 gate);
  all comparisons stay exact f32 on the DVE.
- 4-level pairwise tournament, LSB-first (d4, d3, d2, d1), strict is_gt so
  ties resolve to the lowest window index, matching jnp.argmax.
- The two independent half-chains per chunk are interleaved in program
  order so consecutive same-engine instructions have no data dependence.
- Pool (GPSIMD) on this toolchain only supports add/sub/mult/copy tensor
  ops (no max/is_gt), so comparisons cannot be offloaded there.

Host pre-permute (free): per chunk of 8 channels, partition p=(c8,hi),
free f = a*4096 + b*2048 + ri*1024 + d*512 + s*256 + q*64 + o3*8 + o4.
"""

import os
import sys

import numpy as np

for p in ("/opt/trn_rl_repo", "/opt/pypackages", "/root/.axon_site", "/root/.axon_site/_ro/trn_rl_repo", "/root/.axon_site/_ro/pypackages"):
    if os.path.isdir(p) and p not in sys.path:
        sys.path.append(p)

from concourse import bacc, mybir  # noqa: E402
from concourse.tile import TileContext  # noqa: E402
from concourse.bass_utils import run_bass_kernel_spmd  # noqa: E402

N_CORES = 8
RI = 2
C = 32
D = 16
O = D // 2
NCH = 8                    # channels per chunk
NCHUNK = C // NCH          # 4
SLAB = 1024                # free elems per slab per partition
XF = 8 * SLAB              # 8192

F32 = mybir.dt.float32
BF16 = mybir.dt.bfloat16
U16 = mybir.dt.uint16
I32 = mybir.dt.int32

# engine assignment: comma lists over levels L0,L1,L2,L3 (isgt) / L0,L1,L2 (max)
ISGT_ENG = os.environ.get("K_ISGT_ENG", "v,v,v,v").split(",")
MAX_ENG = os.environ.get("K_MAX_ENG", "v,v,v").split(",")
M_ENG = os.environ.get("K_M_ENG", "ap")  # dve (custom sq+sq) | ap (ACT squares + Pool adds)
SPLIT_DMA = int(os.environ.get("K_SPLIT_DMA", "2"))
# per-chunk count of t-block casts done by DVE tensor_copy instead of ACT
# (balances ACT cast+squares busy vs DVE slack); comma list per chunk
CAST_DVE = [int(x) for x in os.environ.get("K_CAST_DVE", "1,0,1,0").split(",")]
BUFS = int(os.environ.get("K_BUFS", "2"))
BUFS_X = int(os.environ.get("K_BUFS_X", str(BUFS)))
LOOPS = int(os.environ.get("K_LOOPS", "1"))       # whole-kernel idempotent repeats (bench)
REP_DVE = int(os.environ.get("K_REP_DVE", "1"))
REP_ACT = int(os.environ.get("K_REP_ACT", "1"))
REP_GP = int(os.environ.get("K_REP_GP", "1"))
REP_DMA = int(os.environ.get("K_REP_DMA", "1"))
PAYLOAD = os.environ.get("K_PAYLOAD", "bf16")     # bf16 | f32 (f32 = no cast, exact)

_COMPILED = None
_SQSUM = None


def _get_sqsum_op():
    """Register (once per process) the custom DVE op m = in0^2 + in1^2."""
    global _SQSUM
    if _SQSUM is not None:
        return _SQSUM
    from concourse import dve_ops
    from concourse.dve_spec import Spec, Src0, Src1, sq, lower
    from concourse.dve_uop import DveOpSpec

    name = "SQ_SUM_ANT_K"
    if name in dve_ops._SUB_OPCODE_FOR_NAME:
        _SQSUM = next(op for op in dve_ops.OPS if op.name == name)
        return _SQSUM
    spec = Spec(
        body=sq(Src0) + sq(Src1),
        reference=lambda in0, in1, s0, s1, imm2: (
            in0.astype(np.float32) ** 2 + in1.astype(np.float32) ** 2
        ),
    )
    shas = {}
    for ver in ("v3", "v4"):
        uops = lower(spec, ver=ver)
        shas[ver] = DveOpSpec(name=name, opcode=31, uops=uops, rd1_en=True).sha(ver)
    op = dve_ops.DveOp(name, spec, subdim=False, uops_sha=shas)
    row = dve_ops._CUSTOM_DVE_ROW_BASE + len(dve_ops.OPS)
    assert row < 0x20
    dve_ops.OPS.append(op)
    dve_ops.CUSTOM_DVE_SPECS[name] = op.spec
    dve_ops._SUB_OPCODE_FOR_NAME[name] = row
    _SQSUM = op
    return op


def _eng(nc, which):
    return nc.vector if which.strip().startswith("v") else nc.gpsimd


def _build():
    sqsum = _get_sqsum_op()
    nc = bacc.Bacc("TRN2", num_devices=N_CORES)
    x_dram = nc.declare_dram_parameter("x", [NCHUNK, 128, XF], F32, isOutput=False)
    y_dram = nc.declare_dram_parameter(
        "y", [NCHUNK, 128, 512], BF16 if PAYLOAD == "bf16" else F32, isOutput=True
    )

    pay_dt = BF16 if PAYLOAD == "bf16" else F32

    from contextlib import ExitStack
    with TileContext(nc) as tc, ExitStack() as stack:
        pool_x = stack.enter_context(tc.tile_pool(name="xpool", bufs=BUFS_X))
        pool = stack.enter_context(tc.tile_pool(name="sbuf", bufs=BUFS))
        pool_mask = stack.enter_context(tc.tile_pool(name="maskpool", bufs=BUFS))
        for k in [kk for _ in range(LOOPS) for kk in range(NCHUNK)]:
            X = pool_x.tile([128, XF], F32, tag="X")
            for _ in range(REP_DMA):
                step = XF // SPLIT_DMA
                for j in range(SPLIT_DMA):
                    nc.sync.dma_start(
                        out=X[:, j * step:(j + 1) * step],
                        in_=x_dram[k][:, j * step:(j + 1) * step],
                    )

            xtr = X.rearrange("p (t ri f) -> p t ri f", t=4, ri=2)
            # payload copy (bf16, r/i interleaved pairs) and modulus tiles
            if PAYLOAD == "bf16":
                XB = pool.tile([128, XF], BF16, tag="XB")
                # XB physical layout [t, f, ri]; write via strided out AP so
                # in-order (t, ri, f) input lands interleaved
                xb_w = XB.rearrange("p (t f ri) -> p t ri f", t=4, ri=2)
                n_dve = CAST_DVE[k % NCHUNK]
                for h in range(2):
                    # give the last n_dve t-blocks (from the top) to DVE
                    tlo, thi = 2 * h, 2 * h + 2
                    dve_ts = [t for t in range(tlo, thi) if t >= 4 - n_dve]
                    act_ts = [t for t in range(tlo, thi) if t < 4 - n_dve]
                    if act_ts:
                        for _ in range(REP_ACT):
                            nc.scalar.activation(
                                xb_w[:, act_ts[0]:act_ts[-1] + 1],
                                xtr[:, act_ts[0]:act_ts[-1] + 1],
                                mybir.ActivationFunctionType.Copy,
                            )
                    for t in dve_ts:
                        for _ in range(REP_DVE):
                            nc.vector.tensor_copy(xb_w[:, t], xtr[:, t])
                # u32 view: one element per (r,i) pair
                xbp = XB.bitcast(mybir.dt.uint32).rearrange("p (t f) -> p t f", t=4)
            else:
                xbr = xtr
            M = pool.tile([128, 4096], F32, tag="M")  # 4 t-blocks of 1024
            mt = M.rearrange("p (t f) -> p t f", t=4)

            # m = r^2 + i^2 per half
            if M_ENG == "dve":
                # one custom DVE op per half
                for h in range(2):
                    t0 = 2 * h
                    for _ in range(REP_DVE):
                        nc.vector._custom_dve(
                            sqsum,
                            out=mt[:, t0:t0 + 2, :],
                            in0=xtr[:, t0:t0 + 2, 0, :],
                            in1=xtr[:, t0:t0 + 2, 1, :],
                        )
            else:
                # ACT squares (ri-major regrouped) + Pool adds, off the DVE
                for h in range(2):
                    SQ = pool.tile([128, 4096], F32, tag="SQ")
                    for _ in range(REP_ACT):
                        nc.scalar.activation(
                            SQ.rearrange("p (ri b f) -> p b ri f", ri=2, b=2),
                            X[:, h * 4096:(h + 1) * 4096],
                            mybir.ActivationFunctionType.Square,
                        )
                    for _ in range(REP_GP):
                        nc.gpsimd.tensor_tensor(
                            M[:, h * 2048:(h + 1) * 2048],
                            SQ[:, 0:2048], SQ[:, 2048:4096],
                            mybir.AluOpType.add,
                        )

            def pred(t0, n_t, mask_ap, half):
                """payload[t0..t0+n_t-1][0:half] <- [half:2*half] where mask."""
                if mask_ap.dtype == F32:
                    # Pool-made masks are f32 0.0/1.0; cp needs an int dtype —
                    # bitcast (0x0 / 0x3F800000) preserves nonzero-ness.
                    mask_ap = mask_ap.bitcast(I32)
                mk = mask_ap.rearrange("p (t f) -> p t f", t=n_t)
                for _ in range(REP_DVE):
                    if PAYLOAD == "bf16":
                        nc.vector.copy_predicated(
                            xbp[:, t0:t0 + n_t, 0:half], mk,
                            xbp[:, t0:t0 + n_t, half:2 * half],
                        )
                    else:
                        mkb = mk.unsqueeze(2).broadcast_to((128, n_t, 2, half))
                        nc.vector.copy_predicated(
                            xbr[:, t0:t0 + n_t, :, 0:half], mkb,
                            xbr[:, t0:t0 + n_t, :, half:2 * half],
                        )

            def level43(t0, n_t, half, lvl, tag):
                """one d4/d3-style level on blocks t0..t0+n_t-1: [0:half) vs [half:2half)."""
                ei = _eng(nc, ISGT_ENG[lvl])
                # Pool TT ops need matching dtypes: all-f32 is_gt (mask 0.0/1.0)
                maskT = pool_mask.tile(
                    [128, n_t * half], F32 if ei is nc.gpsimd else U16, tag=tag
                )
                if ei is nc.gpsimd:
                    # flat free APs per t-block for gpsimd
                    mkv = maskT.rearrange("p (t f) -> p t f", t=n_t)
                    for t in range(n_t):
                        for _ in range(REP_GP):
                            nc.gpsimd.tensor_tensor(
                                mkv[:, t], mt[:, t0 + t, half:2 * half],
                                mt[:, t0 + t, 0:half], mybir.AluOpType.is_gt,
                            )
                else:
                    for _ in range(REP_DVE):
                        nc.vector.tensor_tensor(
                            maskT.rearrange("p (t f) -> p t f", t=n_t),
                            mt[:, t0:t0 + n_t, half:2 * half],
                            mt[:, t0:t0 + n_t, 0:half],
                            mybir.AluOpType.is_gt,
                        )
                pred(t0, n_t, maskT, half)
                em = _eng(nc, MAX_ENG[lvl])
                if em is nc.gpsimd:
                    for t in range(n_t):
                        for _ in range(REP_GP):
                            nc.gpsimd.tensor_tensor(
                                mt[:, t0 + t, 0:half], mt[:, t0 + t, 0:half],
                                mt[:, t0 + t, half:2 * half], mybir.AluOpType.max,
                            )
                else:
                    for _ in range(REP_DVE):
                        nc.vector.tensor_tensor(
                            mt[:, t0:t0 + n_t, 0:half], mt[:, t0:t0 + n_t, 0:half],
                            mt[:, t0:t0 + n_t, half:2 * half], mybir.AluOpType.max,
                        )

            def level2(t0):
                """b-merge: block t0+1 into t0 on [0:256)."""
                ei = _eng(nc, ISGT_ENG[2])
                mask2 = pool_mask.tile(
                    [128, 256], F32 if ei is nc.gpsimd else U16, tag="mask2"
                )
                for _ in range(REP_GP if ei is nc.gpsimd else REP_DVE):
                    ei.tensor_tensor(
                        mask2[:, :], mt[:, t0 + 1, 0:256], mt[:, t0, 0:256],
                        mybir.AluOpType.is_gt,
                    )
                mk2 = mask2.bitcast(I32) if mask2.dtype == F32 else mask2
                for _ in range(REP_DVE):
                    if PAYLOAD == "bf16":
                        nc.vector.copy_predicated(
                            xbp[:, t0, 0:256], mk2, xbp[:, t0 + 1, 0:256]
                        )
                    else:
                        mk = mk2.unsqueeze(1).broadcast_to((128, 2, 256))
                        nc.vector.copy_predicated(
                            xbr[:, t0, :, 0:256], mk, xbr[:, t0 + 1, :, 0:256]
                        )
                em = _eng(nc, MAX_ENG[2])
                for _ in range(REP_GP if em is nc.gpsimd else REP_DVE):
                    em.tensor_tensor(
                        mt[:, t0, 0:256], mt[:, t0, 0:256], mt[:, t0 + 1, 0:256],
                        mybir.AluOpType.max,
                    )

            # interleave the two halves' chains: consecutive same-engine
            # instructions are independent, filling dependency gaps
            for half, tag in ((512, "mask4"), (256, "mask3")):
                lvl = 0 if half == 512 else 1
                for h in range(2):
                    level43(2 * h, 2, half, lvl, tag + str(h))
            for h in range(2):
                level2(2 * h)

            # D1 (a pairs: t=2 into t=0); no m update
            ei = _eng(nc, ISGT_ENG[3])
            mask1 = pool_mask.tile(
                [128, 256], F32 if ei is nc.gpsimd else U16, tag="mask1"
            )
            for _ in range(REP_GP if ei is nc.gpsimd else REP_DVE):
                ei.tensor_tensor(
                    mask1[:, :], mt[:, 2, 0:256], mt[:, 0, 0:256], mybir.AluOpType.is_gt
                )
            mk1 = mask1.bitcast(I32) if mask1.dtype == F32 else mask1
            for _ in range(REP_DVE):
                if PAYLOAD == "bf16":
                    nc.vector.copy_predicated(xbp[:, 0, 0:256], mk1, xbp[:, 2, 0:256])
                else:
                    mk = mk1.unsqueeze(1).broadcast_to((128, 2, 256))
                    nc.vector.copy_predicated(xbr[:, 0, :, 0:256], mk, xbr[:, 2, :, 0:256])

            # store winners (bf16 y is upcast on host)
            if PAYLOAD == "bf16":
                nc.sync.dma_start(out=y_dram[k], in_=XB[:, 0:512])
            else:
                nc.sync.dma_start(out=y_dram[k], in_=xbr[:, 0, :, 0:256])

    nc.compile()
    return nc


def _get_nc():
    global _COMPILED
    if _COMPILED is None:
        _COMPILED = _build()
    return _COMPILED


def _prep_core(xb: np.ndarray) -> np.ndarray:
    """xb: [2, 32, 16,16,16,16] -> [4, 128, 8192] slab-packed, parity-split."""
    # [ri, chunk, c8, o1, a, o2, b, o3, s, o4, d]
    t = xb.reshape(RI, C // NCH, NCH, O, 2, O, 2, O, 2, O, 2)
    # -> [chunk, a, b, ri, c8, o1, o2, d, s, o3, o4]
    t = t.transpose(1, 4, 6, 0, 2, 3, 5, 10, 8, 7, 9)
    # merge (o1,o2) -> split (hi, q)
    t = t.reshape(C // NCH, 2, 2, RI, NCH, 16, 4, 2, 2, O, O)
    # -> [chunk, c8, hi, a, b, ri, d, s, q, o3, o4]
    t = t.transpose(0, 4, 5, 1, 2, 3, 7, 8, 6, 9, 10)
    return np.ascontiguousarray(t).reshape(C // NCH, 128, XF)


def _post_core(y: np.ndarray) -> np.ndarray:
    """y: [4, 128, 512] -> [2, 32, 8, 8, 8, 8]."""
    if PAYLOAD == "bf16":
        # [chunk, c8, hi, q*o3*o4, ri] (ri innermost: packed pairs)
        yk = y.reshape(C // NCH, NCH, 16, 4 * O * O, RI)
        out = yk.transpose(4, 0, 1, 2, 3).reshape(RI, C, 16, 4 * O * O)
        return out.reshape(RI, C, O, O, O, O)
    # [chunk, c8, hi, ri, q, o3o4]
    yk = y.reshape(C // NCH, NCH, 16, RI, 4, O * O)
    out = yk.transpose(3, 0, 1, 2, 4, 5).reshape(RI, C, 16 * 4, O * O)
    return out.reshape(RI, C, O, O, O, O)


def _run(inputs_x: np.ndarray, trace: bool = False):
    nc = _get_nc()
    in_maps = [{"x": _prep_core(inputs_x[b])} for b in range(N_CORES)]
    last_err = None
    for _attempt in range(3):
        try:
            res = run_bass_kernel_spmd(nc, in_maps, list(range(N_CORES)), trace=trace)
            break
        except Exception as e:  # wedged-device retries
            last_err = e
            if "UNRECOVERABLE" not in str(e) and "UNAVAILABLE" not in str(e):
                raise
    else:
        raise last_err
    outs = np.empty((N_CORES, RI, C, O, O, O, O), dtype=np.float32)
    for b in range(N_CORES):
        outs[b] = _post_core(res.results[b]["y"].astype(np.float32))
    return outs, res


def kernel(input: np.ndarray) -> np.ndarray:
    input = np.asarray(input, dtype=np.float32)
    outs, _ = _run(input)
    return outs
